# revision 1
# baseline (speedup 1.0000x reference)
"""2D Haar DWT (periodization) on Trainium2, data-parallel over 8 NeuronCores.

Input  x: [8, 32, 512, 512] f32  (batch, channel, H, W)
Output (LL, LH, HL, HH), each [8, 32, 256, 256] f32.

Sharding: batch -> 8 cores (one batch element per core, fully local).

Per-core layout: the [32, 512, 512] slice is viewed as 16384 contiguous
rows of 512 floats. Each SBUF partition holds RPP consecutive rows
(RPP/2 H-pairs), so every DMA is a single fully-contiguous block:
  - input tile  [128, RPP*512] f32 (2 MiB for RPP=8)
  - output tile [128, RPP/2*256] per subband (512 KiB for RPP=8)
Butterfly on DVE (all tensor_tensor, fp32 1x):
  stage 1 (H pairs, within-partition contiguous slices):
      S = E + O ; D = E - O
  stage 2 (W pairs, stride-2 views):
      LL = S_e + S_o ; HL = S_e - S_o ; LH = D_e + D_o ; HH = D_e - D_o
The single 0.5 of the separable transform is folded into one in-place
ScalarE pass per output tile right before its store.

The last full tile is split into 4 small subtiles to shorten the
end-of-kernel compute tail behind the final input DMA.
"""

import sys

import numpy as np

if "/opt/trn_rl_repo" not in sys.path:
    sys.path.insert(0, "/opt/trn_rl_repo")

B, C, H, W = 8, 32, 512, 512
ROWS = C * H              # 16384 flat rows per core
RPP = 8                   # input rows per partition (must be even)
TILE_ROWS = 128 * RPP     # 1024
OROWS = ROWS // 2         # 8192 output rows per subband per core
N_CORES = 8

# (row0, nrows) plan: full tiles, then the last tile tapered 4x smaller.
TAPER = 4
PLAN = [(i * TILE_ROWS, TILE_ROWS) for i in range(ROWS // TILE_ROWS - 1)]
PLAN += [
    ((ROWS - TILE_ROWS) + k * (TILE_ROWS // TAPER), TILE_ROWS // TAPER)
    for k in range(TAPER)
]

SUBBANDS = ("ll", "lh", "hl", "hh")

_cache = {}


def _build_program():
    from concourse import bacc, mybir
    from concourse.tile import TileContext

    f32 = mybir.dt.float32
    add = mybir.AluOpType.add
    sub = mybir.AluOpType.subtract

    # Bacc (not raw Bass): its compile() runs generate_event_semaphores(),
    # which splits multi-wait instructions down to the TRN2 limit of one
    # sync wait per instruction — walrus codegen rejects the raw form.
    nc = bacc.Bacc()
    x = nc.dram_tensor("x", [ROWS, W], f32, kind="ExternalInput")
    out = {
        n: nc.dram_tensor(n, [OROWS, W // 2], f32, kind="ExternalOutput")
        for n in SUBBANDS
    }

    with TileContext(nc) as tc, tc.tile_pool(name="p", bufs=3) as pool:
        for r0, nrows in PLAN:
            rpp = nrows // 128        # rows per partition this tile
            jp = rpp // 2             # H-pairs per partition
            # 2D contiguous DMAs everywhere: DRAM side is a plain row
            # slice, SBUF side a flat [128, free] tile (flat iteration
            # orders match elementwise).
            tin = pool.tile([128, rpp * W], f32, tag="tin",
                            padded_shape=[128, RPP * W])
            nc.sync.dma_start(tin[:], x[r0 : r0 + nrows, :])

            t4 = tin.rearrange("p (j o w) -> p j o w", j=jp, o=2)
            e = t4[:, :, 0, :]    # even H rows  [128, jp, 512]
            o = t4[:, :, 1, :]    # odd H rows   [128, jp, 512]

            s = pool.tile([128, jp * W], f32, tag="s",
                          padded_shape=[128, (RPP // 2) * W])
            d = pool.tile([128, jp * W], f32, tag="d",
                          padded_shape=[128, (RPP // 2) * W])
            s3 = s.rearrange("p (j w) -> p j w", j=jp)
            d3 = d.rearrange("p (j w) -> p j w", j=jp)
            nc.vector.tensor_add(out=s3, in0=e, in1=o)
            nc.vector.tensor_sub(out=d3, in0=e, in1=o)

            s4 = s.rearrange("p (j k o) -> p j k o", j=jp, o=2)
            d4 = d.rearrange("p (j k o) -> p j k o", j=jp, o=2)
            se, so = s4[:, :, :, 0], s4[:, :, :, 1]
            de, do = d4[:, :, :, 0], d4[:, :, :, 1]

            ob = {n: pool.tile([128, jp * (W // 2)], f32, tag=n, name=n,
                               padded_shape=[128, (RPP // 2) * (W // 2)])
                  for n in SUBBANDS}
            o3 = {n: ob[n].rearrange("p (j w) -> p j w", j=jp)
                  for n in SUBBANDS}
            nc.vector.tensor_tensor(out=o3["ll"], in0=se, in1=so, op=add)
            nc.vector.tensor_tensor(out=o3["hl"], in0=se, in1=so, op=sub)
            nc.vector.tensor_tensor(out=o3["lh"], in0=de, in1=do, op=add)
            nc.vector.tensor_tensor(out=o3["hh"], in0=de, in1=do, op=sub)

            orow = r0 // 2
            for n in SUBBANDS:
                nc.scalar.mul(ob[n][:], ob[n][:], 0.5)
                nc.sync.dma_start(
                    out[n][orow : orow + nrows // 2, :], ob[n][:]
                )

    nc.finalize()
    return nc


def _run(x, trace=False):
    from concourse.bass_utils import run_bass_kernel_spmd

    if "nc" not in _cache:
        _cache["nc"] = _build_program()
    nc = _cache["nc"]

    x = np.ascontiguousarray(np.asarray(x), dtype=np.float32)
    in_maps = [{"x": x[i].reshape(ROWS, W)} for i in range(N_CORES)]
    res = run_bass_kernel_spmd(nc, in_maps, core_ids=list(range(N_CORES)), trace=trace)
    _cache["last_results"] = res

    outs = []
    for n in ("ll", "lh", "hl", "hh"):
        outs.append(
            np.stack([res.results[i][n].reshape(C, H // 2, W // 2)
                      for i in range(N_CORES)])
        )
    return tuple(outs)


def kernel(x):
    return _run(x, trace=False)

